# revision 8
# baseline (speedup 1.0000x reference)
"""CenterLoss kernel for Trainium2 (Bass/Tile), 8-core data-parallel.

Reference math (B=32768, D=128, C=1000, LAMBD=1.0):
    own = centers[targets]                       # [B, D]
    di  = sum((e - own)^2, -1)                   # [B]
    da  = |e|^2 + |c|^2 - 2 e@c.T                # [B, C]
    loss = sum_{c != y_b} relu(1 + di - da) / (B*(C-1))

Kernel form: the relu argument is
    1 + di_b - |e_b|^2 - |c_c|^2 + 2 e_b.c_c = alpha_b - |c_c|^2 + 2 e_b.c_c
with alpha_b = 1 + sum_d own*(own - 2e) (exactly di - |e|^2 in reals).
At c = y_b the argument is exactly relu(1) = 1, so the masked diagonal is
removed by subtracting B*1.0 from the grand total on the host.

Per core (4096 rows = 32 tiles of 128):
  - PSUM[128b, 500c] = ones_row.T @ (-|c|^2 row)  (rank-1 matmul, start=True)
                     + eT.T @ (2*centers.T)        (K=128 matmul, accumulate)
  - epilogue: relu(psum + alpha_b) summed along c in ONE pass per chunk,
    split between ACT (activation Relu + bias + accum_out) and DVE
    (tensor_scalar add/max + accum_out) so neither engine is the bottleneck.
  - alpha_b from an indirect-DMA gather of centers rows (SWDGE), two DVE ops.
Outputs one partial sum per core; host combines.
"""

import numpy as np

import concourse.bass as bass
import concourse.tile as tile
from concourse import bacc
from concourse import mybir
from concourse.bass_utils import run_bass_kernel_spmd
from concourse.masks import make_identity

B, D, C = 32768, 128, 1000
LAMBD = 1.0
NCORES = 8
BLOC = B // NCORES          # 4096 rows per core
NT = BLOC // 128            # 32 tiles of 128 rows
NCHUNK = 2                  # c chunks per tile
CHUNK = C // NCHUNK         # 500 columns per chunk (<=512, one PSUM bank)
F32 = mybir.dt.float32
# chunk j goes to ACT when (j % 8) < ACT_MOD else DVE
ACT_MOD = 5


def build_program():
    nc = bacc.Bacc("TRN2", target_bir_lowering=False, debug=False,
                   num_devices=NCORES)

    e_dram = nc.dram_tensor("e_loc", [BLOC, D], F32, kind="ExternalInput").ap()
    c_dram = nc.dram_tensor("centers", [C, D], F32, kind="ExternalInput").ap()
    t_dram = nc.dram_tensor("tgt_cols", [128, NT], mybir.dt.int32,
                            kind="ExternalInput").ap()
    out_dram = nc.dram_tensor("partial", [1, 1], F32,
                              kind="ExternalOutput").ap()

    with tile.TileContext(nc) as tc:
        with (
            tc.tile_pool(name="consts", bufs=1) as consts,
            tc.tile_pool(name="cpre", bufs=2) as cpre,
            tc.tile_pool(name="pre_psum", bufs=1, space="PSUM") as pre_psum,
            tc.tile_pool(name="io", bufs=4) as io,
            tc.tile_pool(name="oc", bufs=4) as ocp,
            tc.tile_pool(name="eT", bufs=3) as eTp,
            tc.tile_pool(name="alpha", bufs=6) as alphap,
            tc.tile_pool(name="scratch", bufs=2) as scratch,
            tc.tile_pool(name="mm_psum", bufs=4, space="PSUM") as mm_psum,
            tc.tile_pool(name="tr_psum", bufs=2, space="PSUM") as tr_psum,
        ):
            # ---------------- prologue ----------------
            id1 = consts.tile([128, 128], F32)
            make_identity(nc, id1[:])

            ones_row = consts.tile([1, 128], F32)
            nc.vector.memset(ones_row[:], 1.0)
            ones_col = consts.tile([128, 1], F32)
            nc.vector.memset(ones_col[:], 1.0)

            tgt = consts.tile([128, NT], mybir.dt.int32)
            nc.sync.dma_start(out=tgt[:], in_=t_dram[:])

            # centers.T * 2 -> [128d, 1000c], via PE transposes of row tiles
            cT2 = consts.tile([128, C], F32)
            for i in range(8):
                r0 = i * 128
                r1 = min(r0 + 128, C)
                n = r1 - r0
                ct = cpre.tile([128, 128], F32, tag="ct")
                nc.sync.dma_start(out=ct[:n, :], in_=c_dram[r0:r1, :])
                tp = tr_psum.tile([128, 128], F32, tag="tp")
                nc.tensor.matmul(out=tp[:, :n], lhsT=ct[:n, :],
                                 rhs=id1[:n, :n], start=True, stop=True)
                nc.vector.tensor_scalar_mul(cT2[:, r0:r1], tp[:, :n], 2.0)

            # -|c|^2 as a [1, 1000] row: ones_col.T @ (cT2*cT2) scaled by -1/4
            sq = consts.tile([128, C], F32)
            nc.vector.tensor_tensor(out=sq[:], in0=cT2[:], in1=cT2[:],
                                    op=mybir.AluOpType.mult)
            negcsq = consts.tile([1, C], F32)
            for k in range(NCHUNK):
                cs = k * CHUNK
                csq_ps = pre_psum.tile([1, CHUNK], F32, tag="csqp")
                nc.tensor.matmul(out=csq_ps[:], lhsT=ones_col[:],
                                 rhs=sq[:, cs:cs + CHUNK],
                                 start=True, stop=True)
                nc.scalar.activation(
                    out=negcsq[:, cs:cs + CHUNK], in_=csq_ps[:],
                    func=mybir.ActivationFunctionType.Copy,
                    bias=LAMBD, scale=-0.25)

            # per-engine accumulator columns (one per epilogue chunk op)
            n_act = sum(1 for j in range(NT * NCHUNK) if (j % 8) < ACT_MOD)
            n_dve = NT * NCHUNK - n_act
            zeros_c = consts.tile([128, CHUNK], F32)
            nc.vector.memset(zeros_c[:], 0.0)
            acc_act = consts.tile([128, max(n_act, 1)], F32)
            acc_dve = consts.tile([128, max(n_dve, 1)], F32)
            nc.vector.memset(acc_act[:], 0.0)
            nc.vector.memset(acc_dve[:], 0.0)

            # ---------------- main loop ----------------
            ia = 0
            idv = 0
            for i in range(NT):
                r0 = i * 128
                e_t = io.tile([128, D], F32, tag="e")
                nc.sync.dma_start(out=e_t[:], in_=e_dram[r0:r0 + 128, :])

                oc_t = ocp.tile([128, D], F32, tag="oc")
                nc.gpsimd.indirect_dma_start(
                    out=oc_t[:],
                    out_offset=None,
                    in_=c_dram[:],
                    in_offset=bass.IndirectOffsetOnAxis(ap=tgt[:, i:i + 1],
                                                        axis=0),
                )

                # alpha = sum_d oc*(oc - 2e) == di - |e|^2; LAMBD rides in negcsq
                t1 = scratch.tile([128, D], F32, tag="t1")
                nc.vector.scalar_tensor_tensor(
                    out=t1[:], in0=e_t[:], scalar=-2.0, in1=oc_t[:],
                    op0=mybir.AluOpType.mult, op1=mybir.AluOpType.add)
                t2 = scratch.tile([128, D], F32, tag="tsc")
                nc.vector.tensor_tensor(out=t2[:], in0=t1[:], in1=oc_t[:],
                                        op=mybir.AluOpType.mult)
                alpha = alphap.tile([128, 1], F32, tag="al")
                nc.vector.tensor_reduce(out=alpha[:], in_=t2[:],
                                        axis=mybir.AxisListType.X,
                                        op=mybir.AluOpType.add)

                # eT via PE transpose
                eT_ps = tr_psum.tile([128, 128], F32, tag="tp")
                nc.tensor.matmul(out=eT_ps[:], lhsT=e_t[:], rhs=id1[:],
                                 start=True, stop=True)
                eT = eTp.tile([128, 128], F32, tag="eT")
                nc.scalar.copy(out=eT[:], in_=eT_ps[:])

                for k in range(NCHUNK):
                    cs = k * CHUNK
                    j = i * NCHUNK + k
                    ps = mm_psum.tile([128, CHUNK], F32, tag="ps")
                    nc.tensor.matmul(out=ps[:], lhsT=ones_row[:],
                                     rhs=negcsq[:, cs:cs + CHUNK],
                                     start=True, stop=False)
                    nc.tensor.matmul(out=ps[:], lhsT=eT[:],
                                     rhs=cT2[:, cs:cs + CHUNK],
                                     start=False, stop=True)
                    if (j % 8) < ACT_MOD:
                        so = scratch.tile([128, CHUNK], F32, tag="so_a")
                        nc.scalar.activation(
                            out=so[:], in_=ps[:],
                            func=mybir.ActivationFunctionType.Relu,
                            bias=alpha[:], scale=1.0,
                            accum_out=acc_act[:, ia:ia + 1])
                        ia += 1
                    else:
                        so = scratch.tile([128, CHUNK], F32, tag="so_d")
                        nc.vector.scalar_tensor_tensor(
                            out=so[:], in0=ps[:], scalar=alpha[:],
                            in1=zeros_c[:], op0=mybir.AluOpType.add,
                            op1=mybir.AluOpType.max,
                            accum_out=acc_dve[:, idv:idv + 1])
                        idv += 1

            # ---------------- reduce to one scalar ----------------
            red_a = consts.tile([128, 1], F32)
            red_b = consts.tile([128, 1], F32)
            nc.vector.tensor_reduce(out=red_a[:], in_=acc_act[:],
                                    axis=mybir.AxisListType.X,
                                    op=mybir.AluOpType.add)
            nc.vector.tensor_reduce(out=red_b[:], in_=acc_dve[:],
                                    axis=mybir.AxisListType.X,
                                    op=mybir.AluOpType.add)
            red = consts.tile([128, 1], F32)
            nc.vector.tensor_tensor(out=red[:], in0=red_a[:], in1=red_b[:],
                                    op=mybir.AluOpType.add)
            tot_ps = pre_psum.tile([1, 1], F32, tag="csqp")
            nc.tensor.matmul(out=tot_ps[:], lhsT=red[:], rhs=ones_col[:],
                             start=True, stop=True)
            total = consts.tile([1, 1], F32)
            nc.scalar.copy(out=total[:], in_=tot_ps[:])
            nc.sync.dma_start(out=out_dram[:], in_=total[:])

    nc.compile()
    return nc


_NC_CACHE = None


def _get_nc():
    global _NC_CACHE
    if _NC_CACHE is None:
        _NC_CACHE = build_program()
    return _NC_CACHE


def make_in_maps(embeddings, targets, centers):
    embeddings = np.ascontiguousarray(np.asarray(embeddings, dtype=np.float32))
    centers = np.ascontiguousarray(np.asarray(centers, dtype=np.float32))
    targets = np.asarray(targets).astype(np.int32)
    in_maps = []
    for c in range(NCORES):
        sl = slice(c * BLOC, (c + 1) * BLOC)
        tgt_cols = np.ascontiguousarray(
            targets[sl].reshape(NT, 128).T)  # [128, NT]
        in_maps.append({
            "e_loc": embeddings[sl],
            "centers": centers,
            "tgt_cols": tgt_cols,
        })
    return in_maps


def kernel(embeddings, targets, centers):
    nc = _get_nc()
    in_maps = make_in_maps(embeddings, targets, centers)
    res = run_bass_kernel_spmd(nc, in_maps, list(range(NCORES)))
    total = 0.0
    for r in res.results:
        total += float(r["partial"].reshape(-1)[0])
    loss = (total - B * LAMBD) / (B * (C - 1))
    return np.float32(loss)


# revision 12
# speedup vs baseline: 1.6297x; 1.6297x over previous
"""CenterLoss kernel for Trainium2 (Bass/Tile), 8-core data-parallel.

Reference math (B=32768, D=128, C=1000, LAMBD=1.0):
    own = centers[targets]                       # [B, D]
    di  = sum((e - own)^2, -1)                   # [B]
    da  = |e|^2 + |c|^2 - 2 e@c.T                # [B, C]
    loss = sum_{c != y_b} relu(LAMBD + di - da) / (B*(C-1))

Kernel form: the relu argument is
    LAMBD + di_b - |e_b|^2 - |c_c|^2 + 2 e_b.c_c
  = alpha_b + (LAMBD - |c_c|^2) + 2 e_b.c_c
with alpha_b = sum_d own*(own - 2e)  (== di - |e|^2 in reals).
At c = y_b the argument is relu(LAMBD) = LAMBD exactly, so the masked
diagonal is removed by subtracting B*LAMBD from the grand total on host.

Per core (4096 rows = 32 b-tiles of 128, C split in 2 chunks of 500):
  - PSUM[128b, 500c] = ones2.T @ negcsq2  (K=2 bf16 rank-1: hi/lo split of
                       LAMBD-|c|^2 keeps ~1e-3 absolute accuracy)
                     + eT.T @ (2*centers.T)  (K=128 bf16 matmul)
  - epilogue in ONE pass per chunk: relu(psum + alpha_b) + row-sum, split
    ACT (activation Relu, bias=alpha, accum_out) / DVE (scalar_tensor_tensor
    add alpha, max 0, accum_out) so neither engine bottlenecks.
  - eT tiles via HW DMA-transpose of a host-cast bf16 copy of e (xbar).
  - alpha from an indirect-DMA gather of centers rows, batched 4 b-tiles
    per DVE op chain (t1 = oc-2e, t2 = t1*oc, alpha = sum_d t2).
Outputs one partial sum per core; host combines: (sum - B)/(B*(C-1)).
"""

import numpy as np
import ml_dtypes

import concourse.bass as bass
import concourse.tile as tile
from concourse import bacc
from concourse import mybir
from concourse.bass_utils import run_bass_kernel_spmd
from concourse.masks import make_identity

B, D, C = 32768, 128, 1000
LAMBD = 1.0
NCORES = 8
BLOC = B // NCORES          # 4096 rows per core
NT = BLOC // 128            # 32 tiles of 128 rows
QUAD = 4                    # b-tiles batched per alpha chain
NQ = NT // QUAD             # 8 quads
NCHUNK = 2                  # c chunks per b-tile
CHUNK = C // NCHUNK         # 500 columns (<=512, one PSUM bank)
F32 = mybir.dt.float32
BF16 = mybir.dt.bfloat16
ACT_MOD = 5                 # chunk j -> ACT when (j % 8) < ACT_MOD else DVE


def build_program():
    nc = bacc.Bacc("TRN2", target_bir_lowering=False, debug=False,
                   num_devices=NCORES)

    e_dram = nc.dram_tensor("e_loc", [BLOC, D], F32, kind="ExternalInput").ap()
    ebf_dram = nc.dram_tensor("e_bf", [BLOC, D], BF16,
                              kind="ExternalInput").ap()
    c_dram = nc.dram_tensor("centers", [C, D], F32, kind="ExternalInput").ap()
    t_dram = nc.dram_tensor("tgt_cols", [128, NT], mybir.dt.int32,
                            kind="ExternalInput").ap()
    out_dram = nc.dram_tensor("partial", [1, 1], F32,
                              kind="ExternalOutput").ap()

    with tile.TileContext(nc) as tc:
        with (
            tc.tile_pool(name="consts", bufs=1) as consts,
            tc.tile_pool(name="cpre", bufs=2) as cpre,
            tc.tile_pool(name="pre_psum", bufs=1, space="PSUM") as pre_psum,
            tc.tile_pool(name="io", bufs=3) as io,
            tc.tile_pool(name="eT", bufs=4) as eTp,
            tc.tile_pool(name="scratch", bufs=2) as scratch,
            tc.tile_pool(name="mm_psum", bufs=4, space="PSUM") as mm_psum,
            tc.tile_pool(name="tr_psum", bufs=2, space="PSUM") as tr_psum,
        ):
            # ---------------- prologue ----------------
            id1 = consts.tile([128, 128], F32)
            make_identity(nc, id1[:])

            ones2 = consts.tile([2, 128], BF16)
            nc.vector.memset(ones2[:], 1.0)
            ones_col = consts.tile([128, 1], F32)
            nc.vector.memset(ones_col[:], 1.0)

            tgt = consts.tile([128, NT], mybir.dt.int32)
            nc.sync.dma_start(out=tgt[:], in_=t_dram[:])

            # centers.T: fp32 PE transpose; cT2 = bf16(2*centers.T);
            # sq = (centers.T)^2 in fp32 for an accurate |c|^2.
            cT2 = consts.tile([128, C], BF16)
            sq = consts.tile([128, C], F32)
            for i in range(8):
                r0 = i * 128
                r1 = min(r0 + 128, C)
                n = r1 - r0
                ct = cpre.tile([128, 128], F32, tag="ct")
                nc.sync.dma_start(out=ct[:n, :], in_=c_dram[r0:r1, :])
                tp = tr_psum.tile([128, 128], F32, tag="tp")
                nc.tensor.matmul(out=tp[:, :n], lhsT=ct[:n, :],
                                 rhs=id1[:n, :n], start=True, stop=True)
                nc.vector.tensor_scalar_mul(cT2[:, r0:r1], tp[:, :n], 2.0)
                nc.scalar.activation(
                    out=sq[:, r0:r1], in_=tp[:, :n],
                    func=mybir.ActivationFunctionType.Square)

            # negcsq2: [2, C] bf16 hi/lo split of (LAMBD - |c|^2)
            negf = consts.tile([1, C], F32)
            for k in range(NCHUNK):
                cs = k * CHUNK
                csq_ps = pre_psum.tile([1, CHUNK], F32, tag="csqp")
                nc.tensor.matmul(out=csq_ps[:], lhsT=ones_col[:],
                                 rhs=sq[:, cs:cs + CHUNK],
                                 start=True, stop=True)
                nc.scalar.activation(
                    out=negf[:, cs:cs + CHUNK], in_=csq_ps[:],
                    func=mybir.ActivationFunctionType.Copy,
                    bias=LAMBD, scale=-1.0)
            negcsq2 = consts.tile([2, C], BF16)
            hi_f = consts.tile([1, C], F32)
            lo_f = consts.tile([1, C], F32)
            nc.vector.tensor_copy(out=negcsq2[0:1, :], in_=negf[:])
            nc.vector.tensor_copy(out=hi_f[:], in_=negcsq2[0:1, :])
            nc.vector.tensor_tensor(out=lo_f[:], in0=negf[:], in1=hi_f[:],
                                    op=mybir.AluOpType.subtract)
            lo_bf = consts.tile([1, C], BF16)
            nc.vector.tensor_copy(out=lo_bf[:], in_=lo_f[:])
            nc.gpsimd.dma_start(out=negcsq2[1:2, :], in_=lo_bf[:])

            zeros_c = consts.tile([128, CHUNK], F32)
            nc.vector.memset(zeros_c[:], 0.0)

            n_act = sum(1 for j in range(NT * NCHUNK) if (j % 8) < ACT_MOD)
            n_dve = NT * NCHUNK - n_act
            acc_act = consts.tile([128, max(n_act, 1)], F32)
            acc_dve = consts.tile([128, max(n_dve, 1)], F32)
            nc.vector.memset(acc_act[:], 0.0)
            nc.vector.memset(acc_dve[:], 0.0)

            # ---------------- main loop ----------------
            ia = 0
            idv = 0
            for q in range(NQ):
                r0 = q * QUAD * 128
                # e rows for QUAD b-tiles: [128p, QUADt, 128d]
                e_q = io.tile([128, QUAD, D], F32, tag="e")
                nc.sync.dma_start(
                    out=e_q[:],
                    in_=e_dram[r0:r0 + QUAD * 128, :].rearrange(
                        "(t p) d -> p t d", p=128))
                oc_q = io.tile([128, QUAD, D], F32, tag="oc")
                for t in range(QUAD):
                    nc.gpsimd.indirect_dma_start(
                        out=oc_q[:, t, :],
                        out_offset=None,
                        in_=c_dram[:],
                        in_offset=bass.IndirectOffsetOnAxis(
                            ap=tgt[:, q * QUAD + t:q * QUAD + t + 1], axis=0),
                    )

                # alpha[p, t] = sum_d oc*(oc - 2e)
                t1 = scratch.tile([128, QUAD, D], F32, tag="t1")
                nc.vector.scalar_tensor_tensor(
                    out=t1[:], in0=e_q[:], scalar=-2.0, in1=oc_q[:],
                    op0=mybir.AluOpType.mult, op1=mybir.AluOpType.add)
                t2 = scratch.tile([128, QUAD, D], F32, tag="t2")
                nc.vector.tensor_tensor(out=t2[:], in0=t1[:], in1=oc_q[:],
                                        op=mybir.AluOpType.mult)
                alpha = io.tile([128, QUAD], F32, tag="al")
                nc.vector.tensor_reduce(out=alpha[:], in_=t2[:],
                                        axis=mybir.AxisListType.X,
                                        op=mybir.AluOpType.add)

                for t in range(QUAD):
                    i = q * QUAD + t
                    # eT tile via HW DMA transpose of the bf16 copy of e
                    eT = eTp.tile([128, 128], BF16, tag="eT")
                    nc.sync.dma_start_transpose(
                        out=eT[:], in_=ebf_dram[i * 128:(i + 1) * 128, :])
                    for k in range(NCHUNK):
                        cs = k * CHUNK
                        j = i * NCHUNK + k
                        ps = mm_psum.tile([128, CHUNK], F32, tag="ps")
                        nc.tensor.matmul(out=ps[:], lhsT=ones2[:],
                                         rhs=negcsq2[:, cs:cs + CHUNK],
                                         start=True, stop=False)
                        nc.tensor.matmul(out=ps[:], lhsT=eT[:],
                                         rhs=cT2[:, cs:cs + CHUNK],
                                         start=False, stop=True)
                        if (j % 8) < ACT_MOD:
                            so = scratch.tile([128, CHUNK], F32, tag="so_a")
                            nc.scalar.activation(
                                out=so[:], in_=ps[:],
                                func=mybir.ActivationFunctionType.Relu,
                                bias=alpha[:, t:t + 1], scale=1.0,
                                accum_out=acc_act[:, ia:ia + 1])
                            ia += 1
                        else:
                            so = scratch.tile([128, CHUNK], F32, tag="so_d")
                            nc.vector.scalar_tensor_tensor(
                                out=so[:], in0=ps[:], scalar=alpha[:, t:t + 1],
                                in1=zeros_c[:], op0=mybir.AluOpType.add,
                                op1=mybir.AluOpType.max,
                                accum_out=acc_dve[:, idv:idv + 1])
                            idv += 1

            # ---------------- reduce to one scalar ----------------
            red_a = consts.tile([128, 1], F32)
            red_b = consts.tile([128, 1], F32)
            nc.vector.tensor_reduce(out=red_a[:], in_=acc_act[:],
                                    axis=mybir.AxisListType.X,
                                    op=mybir.AluOpType.add)
            nc.vector.tensor_reduce(out=red_b[:], in_=acc_dve[:],
                                    axis=mybir.AxisListType.X,
                                    op=mybir.AluOpType.add)
            red = consts.tile([128, 1], F32)
            nc.vector.tensor_tensor(out=red[:], in0=red_a[:], in1=red_b[:],
                                    op=mybir.AluOpType.add)
            tot_ps = pre_psum.tile([1, 1], F32, tag="csqp")
            nc.tensor.matmul(out=tot_ps[:], lhsT=red[:], rhs=ones_col[:],
                             start=True, stop=True)
            total = consts.tile([1, 1], F32)
            nc.scalar.copy(out=total[:], in_=tot_ps[:])
            nc.sync.dma_start(out=out_dram[:], in_=total[:])

    nc.compile()
    return nc


_NC_CACHE = None


def _get_nc():
    global _NC_CACHE
    if _NC_CACHE is None:
        _NC_CACHE = build_program()
    return _NC_CACHE


def make_in_maps(embeddings, targets, centers):
    embeddings = np.ascontiguousarray(np.asarray(embeddings, dtype=np.float32))
    e_bf = embeddings.astype(ml_dtypes.bfloat16)
    centers = np.ascontiguousarray(np.asarray(centers, dtype=np.float32))
    targets = np.asarray(targets).astype(np.int32)
    in_maps = []
    for c in range(NCORES):
        sl = slice(c * BLOC, (c + 1) * BLOC)
        tgt_cols = np.ascontiguousarray(
            targets[sl].reshape(NT, 128).T)  # [128, NT]
        in_maps.append({
            "e_loc": embeddings[sl],
            "e_bf": e_bf[sl],
            "centers": centers,
            "tgt_cols": tgt_cols,
        })
    return in_maps


def kernel(embeddings, targets, centers):
    nc = _get_nc()
    in_maps = make_in_maps(embeddings, targets, centers)
    res = run_bass_kernel_spmd(nc, in_maps, list(range(NCORES)))
    total = 0.0
    for r in res.results:
        total += float(r["partial"].reshape(-1)[0])
    loss = (total - B * LAMBD) / (B * (C - 1))
    return np.float32(loss)
